# revision 3
# baseline (speedup 1.0000x reference)
"""Trainium2 Bass kernel for nn_Block_30262339567868 (attention + top-2 MoE block).

Self-contained: takes FULL inputs, shards across 8 NeuronCores internally,
returns the FULL output.

Sharding:
  - Attention: head-parallel (16 heads / 8 cores = 2 heads per core), each core
    produces a partial projection output; host sums partials.
  - MoE: expert-parallel (8 experts / 8 cores), host-side token dispatch
    (gather to per-expert capacity buffers) and gate-weighted scatter-add.
Matmuls run as float32r (tf32-class) except the attention inner (exp(S), V in
bf16). All matmuls use a uniform K=128 contraction (S is zero-padded) — the PE
pays ~200ns per contraction-size switch.
"""

import numpy as np

import concourse.bass as bass
import concourse.mybir as mybir
import concourse.tile as tile
from concourse import bacc
from concourse.bass_utils import run_bass_kernel_spmd
from concourse.masks import make_identity

# Problem shapes (hardcoded per contract)
T = 2048
C = 1024
E = 8
HFF = 4096
NH = 16
HD = 64
NCORES = 8
HPC = NH // NCORES  # heads per core = 2
EPS = 1e-6

F32 = mybir.dt.float32
F32R = mybir.dt.float32r
BF16 = mybir.dt.bfloat16

_nc_cache = {}


# --------------------------------------------------------------------------
# Launch A: attention (head-sharded)
# --------------------------------------------------------------------------

def build_attention():
    if "attn" in _nc_cache:
        return _nc_cache["attn"]
    nc = bacc.Bacc("TRN2", target_bir_lowering=False, debug=False,
                   num_devices=NCORES)

    d_xhatT = nc.dram_tensor("xhatT", [C, T], F32R, kind="ExternalInput")
    d_wqkv = nc.dram_tensor("wqkv", [C, 3 * HPC * HD], F32R, kind="ExternalInput")
    d_wproj = nc.dram_tensor("wproj", [HPC * HD, C], F32R, kind="ExternalInput")
    d_ctab = nc.dram_tensor("ctab", [HPC * HD, T], F32, kind="ExternalInput")
    d_stab = nc.dram_tensor("stab", [HPC * HD, T], F32, kind="ExternalInput")
    # 2 mask tiles of [128, 1024]: offsets (0,128) and (256,384)
    d_mask = nc.dram_tensor("mask", [2, 128, 1024], BF16, kind="ExternalInput")
    d_out = nc.dram_tensor("attn_part", [T, C], F32, kind="ExternalOutput")

    TT = T // 512        # 4 tq chunks
    NTK = T // 128       # 16 tk tiles
    D2 = HPC * HD        # 128
    NKC = C // 128       # 8

    with tile.TileContext(nc) as tc:
        with tc.tile_pool(name="big", bufs=1) as big, \
             tc.tile_pool(name="consts", bufs=1) as consts, \
             tc.tile_pool(name="xstream", bufs=2) as xstream, \
             tc.tile_pool(name="work", bufs=1) as work, \
             tc.tile_pool(name="small", bufs=2) as small, \
             tc.tile_pool(name="estrip", bufs=6) as estrip, \
             tc.tile_pool(name="psA", bufs=2, space="PSUM") as psA, \
             tc.tile_pool(name="psS", bufs=4, space="PSUM") as psS, \
             tc.tile_pool(name="psO", bufs=2, space="PSUM") as psO:

            # ---- DMA inputs ----
            xhatT_r = d_xhatT.ap().rearrange("(ko p) t -> p ko t", p=128)
            wqkv = consts.tile([128, NKC, 3 * D2], F32R)
            nc.sync.dma_start(wqkv[:], d_wqkv.ap().rearrange("(ko p) m -> p ko m", p=128))
            wproj = consts.tile([D2, C], F32R)
            ctab = consts.tile([D2, T], F32)
            stab = consts.tile([D2, T], F32)
            masks = consts.tile([128, 2, 1024], BF16)
            ident = consts.tile([128, 128], F32)

            def load_consts():  # issued after the first x chunk is queued
                nc.sync.dma_start(ctab[:], d_ctab.ap())
                nc.sync.dma_start(stab[:], d_stab.ap())
                nc.sync.dma_start(wproj[:], d_wproj.ap())
                nc.sync.dma_start(masks[:], d_mask.ap().rearrange("m p f -> p m f"))
                make_identity(nc, ident)

            # ---- QKV (K=128 accum groups; f32r), rope fused per chunk ----
            q2 = big.tile([D2, T], F32)
            k2 = big.tile([D2, T], F32)
            v2 = big.tile([D2, T], F32)
            q2s = big.tile([D2, T], F32)  # partition-swapped halves (rope)
            k2s = big.tile([D2, T], F32)
            qhp = [big.tile([128, T], F32R, name=f"qhp{h}") for h in range(HPC)]
            khp = [big.tile([128, T], F32R, name=f"khp{h}") for h in range(HPC)]
            zsrc = work.tile([HD, T], F32, tag="zsrc")
            nc.gpsimd.memset(zsrc[:], 0.0)
            for t_ in qhp + khp:
                nc.vector.tensor_copy(t_[HD:, :], zsrc[:])
            for c in range(TT):
                cs = slice(c * 512, (c + 1) * 512)
                xch = xstream.tile([128, NKC, 512], F32R)
                nc.sync.dma_start(xch[:], xhatT_r[:, :, cs])
                if c == 0:
                    load_consts()
                for g, dst, dsw in ((0, q2, q2s), (1, k2, k2s), (2, v2, None)):
                    ps = psA.tile([128, 512], F32, tag='a')
                    for k in range(NKC):
                        nc.tensor.matmul(
                            ps[:], wqkv[:, k, g * D2:(g + 1) * D2],
                            xch[:, k, :],
                            start=(k == 0), stop=(k == NKC - 1))
                    nc.scalar.copy(dst[:, cs], ps[:])
                    if dsw is not None:
                        # swap 32-partition halves within each 64-row head blk
                        for h in range(HPC):
                            b = h * HD
                            if h == 0:
                                nc.vector.tensor_copy(dsw[b:b + 32, cs], ps[b + 32:b + 64, :])
                                nc.vector.tensor_copy(dsw[b + 32:b + 64, cs], ps[b:b + 32, :])
                            else:
                                nc.scalar.copy(dsw[b:b + 32, cs], ps[b + 32:b + 64, :])
                                nc.scalar.copy(dsw[b + 32:b + 64, cs], ps[b:b + 32, :])
                # rope for this chunk (overlaps later chunks' matmuls)
                for src, ssw, dsts in ((q2, q2s, qhp), (k2, k2s, khp)):
                    t1 = work.tile([D2, 512], F32, tag="rope1")
                    t2 = work.tile([D2, 512], F32, tag="rope2")
                    nc.vector.tensor_mul(t1[:], src[:, cs], ctab[:, cs])
                    nc.vector.tensor_mul(t2[:], ssw[:, cs], stab[:, cs])
                    for h in range(HPC):
                        b = h * HD
                        nc.vector.tensor_add(dsts[h][0:HD, cs], t1[b:b + HD, :],
                                             t2[b:b + HD, :])

            # ---- V transpose -> V' [tk, j, 65] bf16 per head (ones col) ----
            vprime = [big.tile([128, NTK, HD + 1], BF16, name=f"vp{h}")
                      for h in range(HPC)]
            for h in range(HPC):
                nc.any.memset(vprime[h][:, :, HD:HD + 1], 1.0)
            for j in range(NTK):
                pst_full = psA.tile([128, 512], F32, tag='a', name='pst')
                pst = pst_full[:, :128]
                nc.tensor.transpose(pst[:], v2[:, j * 128:(j + 1) * 128], ident[:])
                nc.vector.tensor_copy(vprime[0][:, j, 0:HD], pst[:, 0:HD])
                nc.scalar.copy(vprime[1][:, j, 0:HD], pst[:, HD:2 * HD])

            # ---- attention: SW-pipelined S -> exp -> (mask) -> AV ----
            yhat = big.tile([D2, T], F32R)
            LAG = 3
            for c in range(TT):
                cs = slice(c * 512, (c + 1) * 512)
                for h in range(HPC):
                    njt = 4 * (c + 1)
                    po = psO.tile([HD + 1, 512], F32, tag='o')
                    ets = []

                    def emit_av(j):
                        nc.tensor.matmul(
                            po[:], vprime[h][:, j, :], ets[j][:],
                            start=(j == 0), stop=(j == njt - 1))

                    for j in range(njt):
                        ps = psS.tile([128, 512], F32, tag='s')
                        nc.tensor.matmul(
                            ps[:], khp[h][:, j * 128:(j + 1) * 128],
                            qhp[h][:, cs], start=True, stop=True)
                        et = estrip.tile([128, 512], BF16)
                        nc.scalar.activation(et[:], ps[:],
                                             mybir.ActivationFunctionType.Exp,
                                             scale=float(1.0 / np.sqrt(HD)))
                        m = j - 4 * c
                        if m >= 0:  # diagonal tile: causal mask
                            nc.vector.tensor_mul(et[:], et[:],
                                                 masks[:, m // 2, (m % 2) * 512:
                                                       (m % 2) * 512 + 512])
                        ets.append(et)
                        if j >= LAG:
                            emit_av(j - LAG)
                    for j in range(max(0, njt - LAG), njt):
                        emit_av(j)
                    # normalize: yhat = po[:64] * (1/po[64]) broadcast
                    # (copy denom to partition 0 first: the custom-DVE
                    # reciprocal does not honor input partition offsets)
                    dcp = small.tile([1, 512], F32, tag="dcp")
                    nc.scalar.copy(dcp[:], po[HD:HD + 1, :])
                    rec = small.tile([1, 512], F32, tag="rec")
                    nc.vector.reciprocal_approx_fast(rec[:], dcp[:])
                    rb = small.tile([HD, 512], F32, tag="recb")
                    nc.gpsimd.partition_broadcast(rb[:], rec[:])
                    nc.vector.tensor_mul(yhat[h * HD:(h + 1) * HD, cs],
                                         po[0:HD, :], rb[:])
                # proj for this tq chunk (overlaps next chunk's attention)
                for t in range(4 * c, 4 * (c + 1)):
                    for cc in range(C // 512):
                        pp = psA.tile([128, 512], F32, tag='a')
                        nc.tensor.matmul(pp[:], yhat[:, t * 128:(t + 1) * 128],
                                         wproj[:, cc * 512:(cc + 1) * 512],
                                         start=True, stop=True)
                        ob = small.tile([128, 512], F32, tag="obounce")
                        if (t + cc) % 2 == 0:
                            nc.vector.tensor_copy(ob[:], pp[:])
                        else:
                            nc.scalar.copy(ob[:], pp[:])
                        nc.sync.dma_start(
                            d_out.ap()[t * 128:(t + 1) * 128,
                                       cc * 512:(cc + 1) * 512],
                            ob[:])

    nc.compile()
    _nc_cache["attn"] = nc
    return nc


# --------------------------------------------------------------------------
# Launch B: MoE expert (1 expert per core, host-dispatched tokens)
# --------------------------------------------------------------------------

def _chunks(cap):
    ch = []
    off = 0
    while cap - off >= 512:
        ch.append((off, 512))
        off += 512
    if cap - off:
        ch.append((off, cap - off))
    return ch


def build_moe(cap):
    key = ("moe", cap)
    if key in _nc_cache:
        return _nc_cache[key]
    nc = bacc.Bacc("TRN2", target_bir_lowering=False, debug=False,
                   num_devices=NCORES)

    NKC = C // 128    # 8
    NI = HFF // 128   # 32
    NJ = C // 128     # 8
    CH = _chunks(cap)

    d_xgT = nc.dram_tensor("xgT", [C, cap], F32R, kind="ExternalInput")
    # host-pretiled layouts: [block, 128p, k, 128] with contiguous 4KB+ rows
    d_wg4 = nc.dram_tensor("wg4", [NI, 128, NKC, 128], F32R, kind="ExternalInput")
    d_wu4 = nc.dram_tensor("wu4", [NI, 128, NKC, 128], F32R, kind="ExternalInput")
    d_wd4 = nc.dram_tensor("wd4", [NJ, 128, NI, 128], F32R, kind="ExternalInput")
    d_yT = nc.dram_tensor("yT", [C, cap], F32, kind="ExternalOutput")

    with tile.TileContext(nc) as tc:
        with tc.tile_pool(name="xg", bufs=1) as xgp, \
             tc.tile_pool(name="hsb", bufs=1) as hsbp, \
             tc.tile_pool(name="wload", bufs=3) as wload, \
             tc.tile_pool(name="wdload", bufs=3) as wdload, \
             tc.tile_pool(name="ob", bufs=3) as obp, \
             tc.tile_pool(name="psG", bufs=3, space="PSUM") as psG, \
             tc.tile_pool(name="psY", bufs=2, space="PSUM") as psY:

            xgT_r = d_xgT.ap().rearrange("(ko p) n -> p ko n", p=128)
            xgs = []
            for k in range(NKC):
                xk = xgp.tile([128, cap], F32R, name=f"xg{k}")
                nc.sync.dma_start(xk[:], xgT_r[:, k, :])
                xgs.append(xk)

            hsb = hsbp.tile([128, NI, cap], F32R)

            # Phase 1: h = silu(wg.T @ xg) * (wu.T @ xg), per hidden tile i
            for i in range(NI):
                wgt = wload.tile([128, NKC, 128], F32R, tag="wg")
                nc.sync.dma_start(wgt[:], d_wg4.ap()[i])
                wut = wload.tile([128, NKC, 128], F32R, tag="wu")
                nc.sync.dma_start(wut[:], d_wu4.ap()[i])
                for (off, n) in CH:
                    pg = psG.tile([128, 512], F32, tag="pg")
                    pu = psG.tile([128, 512], F32, tag="pu")
                    for k in range(NKC):
                        nc.tensor.matmul(pg[:, :n], wgt[:, k, :],
                                         xgs[k][:, off:off + n],
                                         start=(k == 0), stop=(k == NKC - 1))
                    for k in range(NKC):
                        nc.tensor.matmul(pu[:, :n], wut[:, k, :],
                                         xgs[k][:, off:off + n],
                                         start=(k == 0), stop=(k == NKC - 1))
                    nc.scalar.activation(hsb[:, i, off:off + n], pg[:, :n],
                                         mybir.ActivationFunctionType.Silu)
                    nc.vector.tensor_mul(hsb[:, i, off:off + n],
                                         hsb[:, i, off:off + n], pu[:, :n])

            # Phase 2: yT[j] = sum_i wd4[j][:, i].T @ h[i]
            for j in range(NJ):
                wdt = wdload.tile([128, NI, 128], F32R, tag="wd")
                nc.sync.dma_start(wdt[:], d_wd4.ap()[j])
                for (off, n) in CH:
                    py = psY.tile([128, 512], F32)
                    for i in range(NI):
                        nc.tensor.matmul(py[:, :n], wdt[:, i, :],
                                         hsb[:, i, off:off + n],
                                         start=(i == 0), stop=(i == NI - 1))
                    ob = obp.tile([128, 512], F32)
                    if j % 2 == 0:
                        nc.vector.tensor_copy(ob[:, :n], py[:, :n])
                    else:
                        nc.scalar.copy(ob[:, :n], py[:, :n])
                    nc.sync.dma_start(
                        d_yT.ap()[j * 128:(j + 1) * 128, off:off + n],
                        ob[:, :n])

    nc.compile()
    _nc_cache[key] = nc
    return nc


# --------------------------------------------------------------------------
# Host orchestration
# --------------------------------------------------------------------------

def _rope_tables():
    inv_freq = 1.0 / (10000.0 ** (np.arange(0, HD, 2, dtype=np.float32) / HD))
    t = np.arange(T, dtype=np.float32)
    freqs = np.einsum("i,j->ij", t, inv_freq).astype(np.float32)   # [T, 32]
    emb = np.concatenate([freqs, freqs], axis=-1)                   # [T, 64]
    cos = np.cos(emb).astype(np.float32)
    sin = np.sin(emb).astype(np.float32)
    cosT = np.ascontiguousarray(cos.T)                              # [64, T]
    # stabA pairs with the partition-swapped operand: d<32 -> -sin, d>=32 -> +sin
    sinA = np.empty((HD, T), np.float32)
    sinA[:32] = -sin.T[:32]
    sinA[32:] = sin.T[32:]
    ctab = np.concatenate([cosT] * HPC, axis=0)                     # [128, T]
    stab = np.concatenate([sinA] * HPC, axis=0)
    return ctab, stab


def _causal_masks():
    # mask[m, p, f] = 1 if (f + 512*... ) — two tiles [128, 1024] covering
    # tk-tile offsets (0,128) and (256,384) relative to the tq chunk start.
    import ml_dtypes
    f = np.arange(512)[None, :]
    p = np.arange(128)[:, None]
    m4 = np.stack([(f >= p + 128 * m) for m in range(4)])            # [4,128,512]
    out = np.concatenate([
        np.concatenate([m4[0], m4[1]], axis=1)[None],                # [128,1024]
        np.concatenate([m4[2], m4[3]], axis=1)[None],
    ]).astype(ml_dtypes.bfloat16)                                    # [2,128,1024]
    return out


def _run(nc, in_maps, trace=False, tmpdir=None):
    return run_bass_kernel_spmd(nc, in_maps, list(range(NCORES)),
                                trace=trace, tmpdir=tmpdir)


def kernel(x, norm1_w, norm2_w, qkv_w, proj_w, router_w, wg, wu, wd,
           _trace=False, _stats=None):
    x = np.asarray(x, np.float32)
    B = x.shape[0]
    xf = x.reshape(T, C)

    # ---- host: rms_norm 1 (norm1_w folded into qkv weights) ----
    ms = np.mean(xf * xf, axis=-1, keepdims=True)
    xhat = xf / np.sqrt(ms + EPS)
    xhatT = np.ascontiguousarray(xhat.T)                    # [C, T]

    ctab, stab = _rope_tables()
    masks = _causal_masks()

    qkv_s = (np.asarray(qkv_w, np.float32) * np.asarray(norm1_w, np.float32)[None, :])
    proj = np.asarray(proj_w, np.float32)

    nc_a = build_attention()
    in_maps = []
    for core in range(NCORES):
        h0 = core * HPC
        rows = []
        for g in range(3):  # q, k, v
            rows.append(qkv_s[g * C + h0 * HD: g * C + (h0 + HPC) * HD, :])
        wqkv_c = np.ascontiguousarray(np.concatenate(rows, axis=0).T)  # [C, 384]
        wproj_c = np.ascontiguousarray(proj[:, h0 * HD:(h0 + HPC) * HD].T)  # [128, C]
        in_maps.append({
            "xhatT": xhatT, "wqkv": wqkv_c, "wproj": wproj_c,
            "ctab": ctab, "stab": stab, "mask": masks,
        })
    res_a = _run(nc_a, in_maps, trace=_trace,
                 tmpdir="/tmp/trace_attn" if _trace else None)
    attn = np.zeros((T, C), np.float32)
    for core in range(NCORES):
        attn += res_a.results[core]["attn_part"]

    xa = xf + attn

    # ---- host: rms_norm 2 + router + top-2 dispatch ----
    ms2 = np.mean(xa * xa, axis=-1, keepdims=True)
    x2 = (xa / np.sqrt(ms2 + EPS)) * np.asarray(norm2_w, np.float32)[None, :]
    logits = x2 @ np.asarray(router_w, np.float32).T        # [T, E]
    topi = np.argsort(-logits, axis=-1)[:, :2]              # [T, 2]
    topv = np.take_along_axis(logits, topi, axis=-1)
    mx = topv.max(axis=-1, keepdims=True)
    ex = np.exp(topv - mx)
    wts = ex / ex.sum(axis=-1, keepdims=True)               # [T, 2]

    idxs, gts = [], []
    for e in range(E):
        sel = np.nonzero((topi == e).any(axis=-1))[0]
        gsel = np.where(topi[sel, 0] == e, wts[sel, 0], wts[sel, 1])
        idxs.append(sel)
        gts.append(gsel.astype(np.float32))
    maxload = max(len(s) for s in idxs)
    cap = max(768, ((maxload + 255) // 256) * 256)

    nc_b = build_moe(cap)
    NI, NJ, NKC = HFF // 128, C // 128, C // 128
    in_maps_b = []
    for e in range(E):
        xgT = np.zeros((C, cap), np.float32)
        xgT[:, :len(idxs[e])] = x2[idxs[e]].T
        wg_e = np.asarray(wg[e], np.float32)
        wu_e = np.asarray(wu[e], np.float32)
        wd_e = np.asarray(wd[e], np.float32)
        in_maps_b.append({
            "xgT": xgT,
            "wg4": np.ascontiguousarray(
                wg_e.reshape(NI, 128, NKC, 128).transpose(0, 3, 2, 1)),
            "wu4": np.ascontiguousarray(
                wu_e.reshape(NI, 128, NKC, 128).transpose(0, 3, 2, 1)),
            "wd4": np.ascontiguousarray(
                wd_e.reshape(NJ, 128, NI, 128).transpose(0, 3, 2, 1)),
        })
    res_b = _run(nc_b, in_maps_b, trace=_trace,
                 tmpdir="/tmp/trace_moe" if _trace else None)

    out = xa.copy()
    for e in range(E):
        yT = res_b.results[e]["yT"]                          # [C, cap]
        n = len(idxs[e])
        out[idxs[e]] += yT[:, :n].T * gts[e][:, None]

    if _stats is not None:
        _stats["attn_ns"] = res_a.exec_time_ns
        _stats["moe_ns"] = res_b.exec_time_ns
        _stats["cap"] = cap
    return out.reshape(B, T, C)



# revision 17
# speedup vs baseline: 1.3544x; 1.3544x over previous
"""Trainium2 Bass kernel for nn_Block_30262339567868 (attention + top-2 MoE block).

Self-contained: takes FULL inputs, shards across 8 NeuronCores internally,
returns the FULL output.

Sharding:
  - Attention: head-parallel (16 heads / 8 cores = 2 heads per core), each core
    produces a partial projection output; host sums partials.
  - MoE: expert-parallel (8 experts / 8 cores), host-side token dispatch
    (gather to per-expert capacity buffers) and gate-weighted scatter-add.
Matmuls run as float32r (tf32-class) except the attention inner (exp(S), V in
bf16). All matmuls use a uniform K=128 contraction (S is zero-padded) — the PE
pays ~200ns per contraction-size switch.
"""

import numpy as np

import concourse.bass as bass
import concourse.mybir as mybir
import concourse.tile as tile
from concourse import bacc
from concourse.bass_utils import run_bass_kernel_spmd
from concourse.masks import make_identity

# Problem shapes (hardcoded per contract)
T = 2048
C = 1024
E = 8
HFF = 4096
NH = 16
HD = 64
NCORES = 8
HPC = NH // NCORES  # heads per core = 2
EPS = 1e-6

F32 = mybir.dt.float32
F32R = mybir.dt.float32r
BF16 = mybir.dt.bfloat16
F8 = mybir.dt.float8e4

# MoE fp8 scale scheme: weights pre-scaled before e4m3 cast; the silu
# input is descaled on-chip (activation scale), the rest is folded into
# the host-side gate weights.
S_W = 32.0   # wg, wd scale
S_U = 16.0   # wu scale == S_H (so h_fp8 = silu(g) * pu directly)
S_H = 16.0

_nc_cache = {}


# --------------------------------------------------------------------------
# Launch A: attention (head-sharded)
# --------------------------------------------------------------------------

def build_attention():
    if "attn" in _nc_cache:
        return _nc_cache["attn"]
    nc = bacc.Bacc("TRN2", target_bir_lowering=False, debug=False,
                   num_devices=NCORES)

    d_xhatT = nc.dram_tensor("xhatT", [C, T], F32R, kind="ExternalInput")
    d_wqkv = nc.dram_tensor("wqkv", [C, 3 * HPC * HD], F32R, kind="ExternalInput")
    d_wproj = nc.dram_tensor("wproj", [HPC * HD, C], F32R, kind="ExternalInput")
    d_ctab = nc.dram_tensor("ctab", [HPC * HD, T], F32, kind="ExternalInput")
    d_stab = nc.dram_tensor("stab", [HPC * HD, T], F32, kind="ExternalInput")
    # additive causal masks for the 4 diagonal k-tiles of a 512-query chunk
    d_mask = nc.dram_tensor("mask", [4, 128, 512], F32R, kind="ExternalInput")
    d_identV = nc.dram_tensor("identV", [128, 128], F32, kind="ExternalInput")
    d_identR = nc.dram_tensor("identR", [128, 128], F32R, kind="ExternalInput")
    d_out = nc.dram_tensor("attn_part", [T, C], F32, kind="ExternalOutput")

    TT = T // 512        # 4 tq chunks
    NTK = T // 128       # 16 tk tiles
    D2 = HPC * HD        # 128
    NKC = C // 128       # 8

    with tile.TileContext(nc) as tc:
        with tc.tile_pool(name="big", bufs=1) as big, \
             tc.tile_pool(name="consts", bufs=1) as consts, \
             tc.tile_pool(name="xstream", bufs=2) as xstream, \
             tc.tile_pool(name="work", bufs=1) as work, \
             tc.tile_pool(name="small", bufs=2) as small, \
             tc.tile_pool(name="estrip", bufs=6) as estrip, \
             tc.tile_pool(name="psA", bufs=2, space="PSUM") as psA, \
             tc.tile_pool(name="psS", bufs=4, space="PSUM") as psS, \
             tc.tile_pool(name="psO", bufs=2, space="PSUM") as psO:

            # ---- constants ----
            xhatT_r = d_xhatT.ap().rearrange("(ko p) t -> p ko t", p=128)
            wqkv = consts.tile([128, NKC, 3 * D2], F32R)
            nc.sync.dma_start(wqkv[:], d_wqkv.ap().rearrange("(ko p) m -> p ko m", p=128))
            wproj = consts.tile([D2, C], F32R)
            ctab = consts.tile([D2, T], F32)
            stab = consts.tile([D2, T], F32)
            maskadd = consts.tile([128, 4, 512], F32R)
            identV = consts.tile([128, 128], F32)
            identR = consts.tile([128, 128], F32R)

            def load_consts():  # issued after the first x k-slice is queued
                nc.sync.dma_start(identR[:], d_identR.ap())
                nc.sync.dma_start(maskadd[:],
                                  d_mask.ap().rearrange("m p f -> p m f"))
                nc.sync.dma_start(identV[:], d_identV.ap())
                nc.sync.dma_start(ctab[:], d_ctab.ap())
                nc.sync.dma_start(stab[:], d_stab.ap())
                nc.sync.dma_start(wproj[:], d_wproj.ap())

            # per-head q/k, real data in rows [0:64], zero-padded [64:128]
            # (uniform K=128 contraction: PE pays ~200ns per size switch)
            qhp = [big.tile([128, T], F32R, name=f"qhp{h}") for h in range(HPC)]
            khp = [big.tile([128, T], F32R, name=f"khp{h}") for h in range(HPC)]
            q2s = big.tile([D2, T], F32)  # partition-swapped halves (rope)
            k2s = big.tile([D2, T], F32)
            v2 = big.tile([D2, T], F32)
            vprime = [big.tile([128, NTK, HD + 1], BF16, name=f"vp{h}")
                      for h in range(HPC)]
            yhat = big.tile([D2, T], F32R)
            for h in range(HPC):
                nc.any.memset(vprime[h][:, :, HD:HD + 1], 1.0)
            # zero the pad rows (hidden under the first x-chunk DMA);
            # memset on f32r is invalid ISA -> bounce zeros via an F32 tile
            zsrc = work.tile([HD, T], F32, tag="zsrc")
            nc.gpsimd.memset(zsrc[:], 0.0)
            nc.vector.tensor_copy(qhp[0][HD:, :], zsrc[:])
            nc.vector.tensor_copy(khp[0][HD:, :], zsrc[:])
            nc.scalar.copy(qhp[1][HD:, :], zsrc[:])
            nc.scalar.copy(khp[1][HD:, :], zsrc[:])

            LAG = 3

            for c in range(TT):
                cs = slice(c * 512, (c + 1) * 512)
                # ---- x chunk DMA, split per k-tile so QKV starts early ----
                xch = xstream.tile([128, NKC, 512], F32R)
                for k in range(NKC):
                    nc.sync.dma_start(xch[:, k, :], xhatT_r[:, k, cs])
                    if c == 0 and k == 0:
                        load_consts()

                # ---- QKV + rope (rope reads PSUM directly) ----
                for g, dsw, dsts in ((0, q2s, qhp), (1, k2s, khp),
                                     (2, None, None)):
                    ps = psA.tile([128, 512], F32, tag='a')
                    for k in range(NKC):
                        nc.tensor.matmul(
                            ps[:], wqkv[:, k, g * D2:(g + 1) * D2],
                            xch[:, k, :],
                            start=(k == 0), stop=(k == NKC - 1))
                    if dsw is None:
                        nc.scalar.copy(v2[:, cs], ps[:])
                        continue
                    # swap 32-partition halves within each 64-row head block
                    # (Pool/GpSimd cannot read PSUM: vector/scalar only)
                    for h in range(HPC):
                        b = h * HD
                        eng = nc.vector if h == 0 else nc.scalar
                        if eng is nc.vector:
                            eng.tensor_copy(dsw[b:b + 32, cs], ps[b + 32:b + 64, :])
                            eng.tensor_copy(dsw[b + 32:b + 64, cs], ps[b:b + 32, :])
                        else:
                            eng.copy(dsw[b:b + 32, cs], ps[b + 32:b + 64, :])
                            eng.copy(dsw[b + 32:b + 64, cs], ps[b:b + 32, :])
                    t1 = work.tile([D2, 512], F32, tag="rope1")
                    t2 = work.tile([D2, 512], F32, tag="rope2")
                    nc.vector.tensor_mul(t1[:], ps[:], ctab[:, cs])
                    nc.vector.tensor_mul(t2[:], dsw[:, cs], stab[:, cs])
                    for h in range(HPC):
                        b = h * HD
                        nc.vector.tensor_add(dsts[h][0:HD, cs], t1[b:b + HD, :],
                                             t2[b:b + HD, :])

                # ---- V' for this chunk's 4 k-tiles ----
                for j in range(4 * c, 4 * (c + 1)):
                    pst_full = psA.tile([128, 512], F32, tag='a', name='pst')
                    pst = pst_full[:, :128]
                    nc.tensor.transpose(pst[:], v2[:, j * 128:(j + 1) * 128],
                                        identV[:])
                    nc.vector.tensor_copy(vprime[0][:, j, 0:HD], pst[:, 0:HD])
                    nc.vector.tensor_copy(vprime[1][:, j, 0:HD],
                                          pst[:, HD:2 * HD])

                # ---- attention: SW-pipelined S(+mask) -> exp -> AV ----
                for h in range(HPC):
                    njt = 4 * (c + 1)
                    hb = h * HD
                    po = psO.tile([HD + 1, 512], F32, tag='o')
                    ets = []

                    def emit_av(j):
                        nc.tensor.matmul(
                            po[:], vprime[h][:, j, :], ets[j][:],
                            start=(j == 0), stop=(j == njt - 1))

                    for j in range(njt):
                        ps = psS.tile([128, 512], F32, tag='s')
                        m = j - 4 * c
                        nc.tensor.matmul(
                            ps[:], khp[h][:, j * 128:(j + 1) * 128],
                            qhp[h][:, cs], start=True, stop=(m < 0))
                        if m >= 0:  # diagonal tile: additive causal mask
                            nc.tensor.matmul(ps[:], identR[:],
                                             maskadd[:, m, :],
                                             start=False, stop=True)
                        et = estrip.tile([128, 512], BF16)
                        nc.scalar.activation(et[:], ps[:],
                                             mybir.ActivationFunctionType.Exp,
                                             scale=float(1.0 / np.sqrt(HD)))
                        ets.append(et)
                        if j >= LAG:
                            emit_av(j - LAG)
                    for j in range(max(0, njt - LAG), njt):
                        emit_av(j)
                    # normalize: yhat = po[:64] * (1/po[64]) broadcast
                    # (copy denom to partition 0 first: the custom-DVE
                    # reciprocal does not honor input partition offsets)
                    dcp = small.tile([1, 512], F32, tag="dcp")
                    nc.scalar.copy(dcp[:], po[HD:HD + 1, :])
                    rec = small.tile([1, 512], F32, tag="rec")
                    nc.vector.reciprocal_approx_fast(rec[:], dcp[:])
                    rb = small.tile([HD, 512], F32, tag="recb")
                    nc.gpsimd.partition_broadcast(rb[:], rec[:])
                    nc.vector.tensor_mul(yhat[hb:hb + HD, cs],
                                         po[0:HD, :], rb[:])
                # ---- proj for this tq chunk ----
                for t in range(4 * c, 4 * (c + 1)):
                    for cc in range(C // 512):
                        pp = psA.tile([128, 512], F32, tag='a')
                        nc.tensor.matmul(pp[:], yhat[:, t * 128:(t + 1) * 128],
                                         wproj[:, cc * 512:(cc + 1) * 512],
                                         start=True, stop=True)
                        ob = small.tile([128, 512], F32, tag="obounce")
                        if (t + cc) % 2 == 0:
                            nc.vector.tensor_copy(ob[:], pp[:])
                        else:
                            nc.scalar.copy(ob[:], pp[:])
                        nc.sync.dma_start(
                            d_out.ap()[t * 128:(t + 1) * 128,
                                       cc * 512:(cc + 1) * 512],
                            ob[:])

    nc.compile()
    _nc_cache["attn"] = nc
    return nc


# --------------------------------------------------------------------------
# Launch B: MoE expert (1 expert per core, host-dispatched tokens)
# --------------------------------------------------------------------------

def _chunks(cap):
    ch = []
    off = 0
    while cap - off >= 512:
        ch.append((off, 512))
        off += 512
    if cap - off:
        ch.append((off, cap - off))
    return ch


def build_moe(cap):
    key = ("moe", cap)
    if key in _nc_cache:
        return _nc_cache[key]
    nc = bacc.Bacc("TRN2", target_bir_lowering=False, debug=False,
                   num_devices=NCORES)

    NKC = C // 128    # 8
    NI = HFF // 128   # 32
    NJ = C // 128     # 8
    CH = _chunks(cap)
    DR = mybir.MatmulPerfMode.DoubleRow

    d_xgT = nc.dram_tensor("xgT", [C, cap], F8, kind="ExternalInput")
    # host-pretiled layouts: [block, 128p, k, 128] with contiguous rows;
    # weights pre-scaled by S_W (wg, wd) / S_U (wu) and cast to fp8e4.
    d_wg4 = nc.dram_tensor("wg4", [NI, 128, NKC, 128], F8, kind="ExternalInput")
    d_wu4 = nc.dram_tensor("wu4", [NI, 128, NKC, 128], F8, kind="ExternalInput")
    d_wd4 = nc.dram_tensor("wd4", [NJ, 128, NI, 128], F8, kind="ExternalInput")
    d_yT = nc.dram_tensor("yT", [C, cap], F32, kind="ExternalOutput")

    with tile.TileContext(nc) as tc:
        with tc.tile_pool(name="xg", bufs=1) as xgp, \
             tc.tile_pool(name="hsb", bufs=1) as hsbp, \
             tc.tile_pool(name="hst", bufs=3) as hstp, \
             tc.tile_pool(name="wload", bufs=3) as wload, \
             tc.tile_pool(name="wdload", bufs=3) as wdload, \
             tc.tile_pool(name="ob", bufs=3) as obp, \
             tc.tile_pool(name="psG", bufs=3, space="PSUM") as psG, \
             tc.tile_pool(name="psY", bufs=2, space="PSUM") as psY:

            # xg as one [128, k, n] tile so DoubleRow can take k-pair slices
            xg = xgp.tile([128, NKC, cap], F8)
            nc.sync.dma_start(xg[:], d_xgT.ap().rearrange(
                "(ko p) n -> p ko n", p=128))

            hsb = hsbp.tile([128, NI, cap], F8)

            # Phase 1: h = silu(wg.T@xg) * (wu.T@xg); fp8 DoubleRow over
            # k-tile pairs (contraction 256/instr).
            for i in range(NI):
                wgt = wload.tile([128, NKC, 128], F8, tag="wg")
                nc.sync.dma_start(wgt[:], d_wg4.ap()[i])
                wut = wload.tile([128, NKC, 128], F8, tag="wu")
                nc.sync.dma_start(wut[:], d_wu4.ap()[i])
                for (off, n) in CH:
                    pg = psG.tile([128, 512], F32, tag="pg")
                    pu = psG.tile([128, 512], F32, tag="pu")
                    for t in range(NKC // 2):
                        nc.tensor.matmul(pg[:, :n], wgt[:, 2 * t:2 * t + 2, :],
                                         xg[:, 2 * t:2 * t + 2, off:off + n],
                                         start=(t == 0), stop=(t == NKC // 2 - 1),
                                         perf_mode=DR)
                    for t in range(NKC // 2):
                        nc.tensor.matmul(pu[:, :n], wut[:, 2 * t:2 * t + 2, :],
                                         xg[:, 2 * t:2 * t + 2, off:off + n],
                                         start=(t == 0), stop=(t == NKC // 2 - 1),
                                         perf_mode=DR)
                    hs = hstp.tile([128, 512], F32)
                    nc.scalar.activation(hs[:, :n], pg[:, :n],
                                         mybir.ActivationFunctionType.Silu,
                                         scale=float(1.0 / S_W))
                    # hsb = silu(g) * (S_U*u) = S_H*h  (S_H == S_U)
                    nc.vector.tensor_mul(hsb[:, i, off:off + n],
                                         hs[:, :n], pu[:, :n])

            # Phase 2: yT[j] = sum_i wd4[j][:, i].T @ h[i]; DR over i-pairs
            for j in range(NJ):
                wdt = wdload.tile([128, NI, 128], F8, tag="wd")
                nc.sync.dma_start(wdt[:], d_wd4.ap()[j])
                for (off, n) in CH:
                    py = psY.tile([128, 512], F32)
                    for t in range(NI // 2):
                        nc.tensor.matmul(py[:, :n], wdt[:, 2 * t:2 * t + 2, :],
                                         hsb[:, 2 * t:2 * t + 2, off:off + n],
                                         start=(t == 0), stop=(t == NI // 2 - 1),
                                         perf_mode=DR)
                    ob = obp.tile([128, 512], F32)
                    if j % 2 == 0:
                        nc.vector.tensor_copy(ob[:, :n], py[:, :n])
                    else:
                        nc.scalar.copy(ob[:, :n], py[:, :n])
                    nc.sync.dma_start(
                        d_yT.ap()[j * 128:(j + 1) * 128, off:off + n],
                        ob[:, :n])

    nc.compile()
    _nc_cache[key] = nc
    return nc


# --------------------------------------------------------------------------
# Host orchestration
# --------------------------------------------------------------------------

def _rope_tables():
    inv_freq = 1.0 / (10000.0 ** (np.arange(0, HD, 2, dtype=np.float32) / HD))
    t = np.arange(T, dtype=np.float32)
    freqs = np.einsum("i,j->ij", t, inv_freq).astype(np.float32)   # [T, 32]
    emb = np.concatenate([freqs, freqs], axis=-1)                   # [T, 64]
    cos = np.cos(emb).astype(np.float32)
    sin = np.sin(emb).astype(np.float32)
    cosT = np.ascontiguousarray(cos.T)                              # [64, T]
    # stabA pairs with the partition-swapped operand: d<32 -> -sin, d>=32 -> +sin
    sinA = np.empty((HD, T), np.float32)
    sinA[:32] = -sin.T[:32]
    sinA[32:] = sin.T[32:]
    ctab = np.concatenate([cosT] * HPC, axis=0)                     # [128, T]
    stab = np.concatenate([sinA] * HPC, axis=0)
    return ctab, stab


def _causal_masks():
    # additive mask[m, p, f] = 0 where query f sees key (p + 128*m) within
    # the diagonal 512-block, else -1e30 (absorbs s in fp32, exp -> 0).
    f = np.arange(512)[None, :]
    p = np.arange(128)[:, None]
    m4 = np.stack([np.where(f >= p + 128 * m, 0.0, -1e30)
                   for m in range(4)]).astype(np.float32)            # [4,128,512]
    return m4


def _run(nc, in_maps, trace=False, tmpdir=None):
    return run_bass_kernel_spmd(nc, in_maps, list(range(NCORES)),
                                trace=trace, tmpdir=tmpdir)


def kernel(x, norm1_w, norm2_w, qkv_w, proj_w, router_w, wg, wu, wd,
           _trace=False, _stats=None):
    x = np.asarray(x, np.float32)
    B = x.shape[0]
    xf = x.reshape(T, C)

    # ---- host: rms_norm 1 (norm1_w folded into qkv weights) ----
    ms = np.mean(xf * xf, axis=-1, keepdims=True)
    xhat = xf / np.sqrt(ms + EPS)
    xhatT = np.ascontiguousarray(xhat.T)                    # [C, T]

    ctab, stab = _rope_tables()
    masks = _causal_masks()

    qkv_s = (np.asarray(qkv_w, np.float32) * np.asarray(norm1_w, np.float32)[None, :])
    proj = np.asarray(proj_w, np.float32)

    nc_a = build_attention()
    ident = np.eye(128, dtype=np.float32)
    in_maps = []
    for core in range(NCORES):
        h0 = core * HPC
        rows = []
        for g in range(3):  # q, k, v
            rows.append(qkv_s[g * C + h0 * HD: g * C + (h0 + HPC) * HD, :])
        wqkv_c = np.ascontiguousarray(np.concatenate(rows, axis=0).T)  # [C, 384]
        wproj_c = np.ascontiguousarray(proj[:, h0 * HD:(h0 + HPC) * HD].T)  # [128, C]
        in_maps.append({
            "xhatT": xhatT, "wqkv": wqkv_c, "wproj": wproj_c,
            "ctab": ctab, "stab": stab, "mask": masks,
            "identV": ident, "identR": ident,
        })
    res_a = _run(nc_a, in_maps, trace=_trace,
                 tmpdir="/tmp/trace_attn" if _trace else None)
    attn = np.zeros((T, C), np.float32)
    for core in range(NCORES):
        attn += res_a.results[core]["attn_part"]

    xa = xf + attn

    # ---- host: rms_norm 2 + router + top-2 dispatch ----
    ms2 = np.mean(xa * xa, axis=-1, keepdims=True)
    x2 = (xa / np.sqrt(ms2 + EPS)) * np.asarray(norm2_w, np.float32)[None, :]
    logits = x2 @ np.asarray(router_w, np.float32).T        # [T, E]
    topi = np.argsort(-logits, axis=-1)[:, :2]              # [T, 2]
    topv = np.take_along_axis(logits, topi, axis=-1)
    mx = topv.max(axis=-1, keepdims=True)
    ex = np.exp(topv - mx)
    wts = ex / ex.sum(axis=-1, keepdims=True)               # [T, 2]

    idxs, gts = [], []
    for e in range(E):
        sel = np.nonzero((topi == e).any(axis=-1))[0]
        gsel = np.where(topi[sel, 0] == e, wts[sel, 0], wts[sel, 1])
        idxs.append(sel)
        gts.append(gsel.astype(np.float32))
    maxload = max(len(s) for s in idxs)
    cap = max(256, ((maxload + 127) // 128) * 128)

    import ml_dtypes
    F8NP = ml_dtypes.float8_e4m3

    nc_b = build_moe(cap)
    NI, NJ, NKC = HFF // 128, C // 128, C // 128
    in_maps_b = []
    for e in range(E):
        xgT = np.zeros((C, cap), F8NP)
        xgT[:, :len(idxs[e])] = x2[idxs[e]].T.astype(F8NP)
        wg_e = np.asarray(wg[e], np.float32) * S_W
        wu_e = np.asarray(wu[e], np.float32) * S_U
        wd_e = np.asarray(wd[e], np.float32) * S_W
        in_maps_b.append({
            "xgT": xgT,
            "wg4": np.ascontiguousarray(
                wg_e.reshape(NI, 128, NKC, 128).transpose(0, 3, 2, 1)
            ).astype(F8NP),
            "wu4": np.ascontiguousarray(
                wu_e.reshape(NI, 128, NKC, 128).transpose(0, 3, 2, 1)
            ).astype(F8NP),
            "wd4": np.ascontiguousarray(
                wd_e.reshape(NJ, 128, NI, 128).transpose(0, 3, 2, 1)
            ).astype(F8NP),
        })
    res_b = _run(nc_b, in_maps_b, trace=_trace,
                 tmpdir="/tmp/trace_moe" if _trace else None)

    out = xa.copy()
    for e in range(E):
        yT = res_b.results[e]["yT"]                          # [C, cap]
        n = len(idxs[e])
        out[idxs[e]] += yT[:, :n].T * (gts[e] / (S_W * S_H))[:, None]

    if _stats is not None:
        _stats["attn_ns"] = res_a.exec_time_ns
        _stats["moe_ns"] = res_b.exec_time_ns
        _stats["cap"] = cap
    return out.reshape(B, T, C)



# revision 28
# speedup vs baseline: 1.6384x; 1.2097x over previous
"""Trainium2 Bass kernel for nn_Block_30262339567868 (attention + top-2 MoE block).

Self-contained: takes FULL inputs, shards across 8 NeuronCores internally,
returns the FULL output.

Sharding:
  - Attention: head-parallel (16 heads / 8 cores = 2 heads per core), each core
    produces a partial projection output; host sums partials.
  - MoE: expert-parallel (8 experts / 8 cores), host-side token dispatch
    (gather to per-expert capacity buffers) and gate-weighted scatter-add.
Matmuls run as float32r (tf32-class) except the attention inner (exp(S), V in
bf16). All matmuls use a uniform K=128 contraction (S is zero-padded) — the PE
pays ~200ns per contraction-size switch.
"""

import numpy as np

import concourse.bass as bass
import concourse.mybir as mybir
import concourse.tile as tile
from concourse import bacc
from concourse.bass_utils import run_bass_kernel_spmd
from concourse.masks import make_identity

# Problem shapes (hardcoded per contract)
T = 2048
C = 1024
E = 8
HFF = 4096
NH = 16
HD = 64
NCORES = 8
HPC = NH // NCORES  # heads per core = 2
EPS = 1e-6

F32 = mybir.dt.float32
F32R = mybir.dt.float32r
BF16 = mybir.dt.bfloat16
F8 = mybir.dt.float8e4

# MoE fp8 scale scheme: weights pre-scaled before e4m3 cast; the silu
# input is descaled on-chip (activation scale), the rest is folded into
# the host-side gate weights.
S_W = 32.0   # wg, wd scale
S_U = 16.0   # wu scale == S_H (so h_fp8 = silu(g) * pu directly)
S_H = 16.0

_nc_cache = {}


# --------------------------------------------------------------------------
# Launch A: attention (head-sharded)
# --------------------------------------------------------------------------

def build_attention():
    if "attn" in _nc_cache:
        return _nc_cache["attn"]
    nc = bacc.Bacc("TRN2", target_bir_lowering=False, debug=False,
                   num_devices=NCORES)

    d_xhatT = nc.dram_tensor("xhatT", [C, T], F32R, kind="ExternalInput")
    d_wqkv = nc.dram_tensor("wqkv", [C, 3 * HPC * HD], F32R, kind="ExternalInput")
    d_wproj = nc.dram_tensor("wproj", [HPC * HD, C], F32R, kind="ExternalInput")
    d_ctab = nc.dram_tensor("ctab", [HPC * HD, T], F32, kind="ExternalInput")
    d_stab = nc.dram_tensor("stab", [HPC * HD, T], F32, kind="ExternalInput")
    # additive causal masks for the 4 diagonal k-tiles of a 512-query chunk
    d_mask = nc.dram_tensor("mask", [4, 128, 512], F32R, kind="ExternalInput")
    d_identV = nc.dram_tensor("identV", [128, 128], F32, kind="ExternalInput")
    d_identR = nc.dram_tensor("identR", [128, 128], F32R, kind="ExternalInput")
    d_out = nc.dram_tensor("attn_part", [T, C], F32, kind="ExternalOutput")

    TT = T // 512        # 4 tq chunks
    NTK = T // 128       # 16 tk tiles
    D2 = HPC * HD        # 128
    NKC = C // 128       # 8

    with tile.TileContext(nc) as tc:
        with tc.tile_pool(name="big", bufs=1) as big, \
             tc.tile_pool(name="consts", bufs=1) as consts, \
             tc.tile_pool(name="xstream", bufs=2) as xstream, \
             tc.tile_pool(name="work", bufs=1) as work, \
             tc.tile_pool(name="small", bufs=2) as small, \
             tc.tile_pool(name="ostage", bufs=2) as ostage, \
             tc.tile_pool(name="estrip", bufs=10) as estrip, \
             tc.tile_pool(name="psA", bufs=2, space="PSUM") as psA, \
             tc.tile_pool(name="psS", bufs=4, space="PSUM") as psS, \
             tc.tile_pool(name="psO", bufs=1, space="PSUM") as psO:

            # ---- constants ----
            xhatT_r = d_xhatT.ap().rearrange("(ko p) t -> p ko t", p=128)
            wqkv = consts.tile([128, NKC, 3 * D2], F32R)
            nc.sync.dma_start(wqkv[:], d_wqkv.ap().rearrange("(ko p) m -> p ko m", p=128))
            wproj = consts.tile([D2, C], F32R)
            ctab = consts.tile([D2, T], F32)
            stab = consts.tile([D2, T], F32)
            maskadd = consts.tile([128, 4, 512], F32R)
            identV = consts.tile([128, 128], F32)
            identR = consts.tile([128, 128], F32R)

            def load_consts():  # queued after chunk 0's x data
                nc.sync.dma_start(ctab[:], d_ctab.ap())
                nc.sync.dma_start(stab[:], d_stab.ap())
                nc.sync.dma_start(identV[:], d_identV.ap())
                nc.sync.dma_start(identR[:], d_identR.ap())
                nc.sync.dma_start(maskadd[:],
                                  d_mask.ap().rearrange("m p f -> p m f"))

            # per-head q/k, real data in rows [0:64], zero-padded [64:128]
            # (uniform K=128 contraction: PE pays ~200ns per size switch)
            qhp = [big.tile([128, T], F32R, name=f"qhp{h}") for h in range(HPC)]
            khp = [big.tile([128, T], F32R, name=f"khp{h}") for h in range(HPC)]
            q2s = big.tile([D2, T], F32)  # partition-swapped halves (rope)
            k2s = big.tile([D2, T], F32)
            v2 = big.tile([D2, T], F32)
            vprime = [big.tile([128, NTK, HD + 1], BF16, name=f"vp{h}")
                      for h in range(HPC)]
            yhat = big.tile([D2, T], F32R)
            for h in range(HPC):
                nc.any.memset(vprime[h][:, :, HD:HD + 1], 1.0)
            # zero the pad rows (hidden under the first x-chunk DMA);
            # memset on f32r is invalid ISA -> bounce zeros via an F32 tile
            zsrc = work.tile([HD, T], F32, tag="zsrc")
            nc.gpsimd.memset(zsrc[:], 0.0)
            nc.vector.tensor_copy(qhp[0][HD:, :], zsrc[:])
            nc.vector.tensor_copy(khp[0][HD:, :], zsrc[:])
            nc.scalar.copy(qhp[1][HD:, :], zsrc[:])
            nc.scalar.copy(khp[1][HD:, :], zsrc[:])

            LAG = 3
            pending_proj = []
            # output staged per chunk: [128, 4 t-tiles, C] -> one 2MB DMA
            outT_r = d_out.ap().rearrange("(b tt p) c -> b p tt c",
                                          tt=4, p=128)

            def emit_proj(c):
                obc = ostage.tile([128, 4, C], F32)
                for t in range(4 * c, 4 * (c + 1)):
                    tl = t - 4 * c
                    for cc in range(C // 512):
                        pp = psA.tile([128, 512], F32, tag='a')
                        nc.tensor.matmul(pp[:], yhat[:, t * 128:(t + 1) * 128],
                                         wproj[:, cc * 512:(cc + 1) * 512],
                                         start=True, stop=True)
                        if (t + cc) % 2 == 0:
                            nc.vector.tensor_copy(
                                obc[:, tl, cc * 512:(cc + 1) * 512], pp[:])
                        else:
                            nc.scalar.copy(
                                obc[:, tl, cc * 512:(cc + 1) * 512], pp[:])
                nc.sync.dma_start(outT_r[c], obc[:])

            for c in range(TT):
                cs = slice(c * 512, (c + 1) * 512)
                # ---- x chunk DMA (halved for chunk 0: earlier QKV start) ----
                xch = xstream.tile([128, NKC, 512], F32R)
                if c == 0:
                    nc.sync.dma_start(xch[:, 0:4, :], xhatT_r[:, 0:4, cs])
                    nc.sync.dma_start(xch[:, 4:8, :], xhatT_r[:, 4:8, cs])
                    load_consts()
                elif c == 1:
                    nc.sync.dma_start(xch[:], xhatT_r[:, :, cs])
                    nc.sync.dma_start(wproj[:], d_wproj.ap())
                else:
                    nc.sync.dma_start(xch[:], xhatT_r[:, :, cs])

                # ---- QKV + rope (rope reads PSUM directly) ----
                for g, dsw, dsts in ((0, q2s, qhp), (1, k2s, khp),
                                     (2, None, None)):
                    ps = psA.tile([128, 512], F32, tag='a')
                    for k in range(NKC):
                        nc.tensor.matmul(
                            ps[:], wqkv[:, k, g * D2:(g + 1) * D2],
                            xch[:, k, :],
                            start=(k == 0), stop=(k == NKC - 1))
                    if dsw is None:
                        nc.scalar.copy(v2[:, cs], ps[:])
                        continue
                    # swap 32-partition halves within each 64-row head block
                    # (Pool/GpSimd cannot read PSUM: vector/scalar only)
                    for h in range(HPC):
                        b = h * HD
                        eng = nc.vector if h == 0 else nc.scalar
                        if eng is nc.vector:
                            eng.tensor_copy(dsw[b:b + 32, cs], ps[b + 32:b + 64, :])
                            eng.tensor_copy(dsw[b + 32:b + 64, cs], ps[b:b + 32, :])
                        else:
                            eng.copy(dsw[b:b + 32, cs], ps[b + 32:b + 64, :])
                            eng.copy(dsw[b + 32:b + 64, cs], ps[b:b + 32, :])
                    t1 = work.tile([D2, 512], F32, tag="rope1")
                    t2 = work.tile([D2, 512], F32, tag="rope2")
                    nc.vector.tensor_mul(t1[:], ps[:], ctab[:, cs])
                    nc.vector.tensor_mul(t2[:], dsw[:, cs], stab[:, cs])
                    for h in range(HPC):
                        b = h * HD
                        nc.vector.tensor_add(dsts[h][0:HD, cs], t1[b:b + HD, :],
                                             t2[b:b + HD, :])

                # ---- V' for this chunk's 4 k-tiles ----
                for j in range(4 * c, 4 * (c + 1)):
                    pst_full = psA.tile([128, 512], F32, tag='a', name='pst')
                    pst = pst_full[:, :128]
                    nc.tensor.transpose(pst[:], v2[:, j * 128:(j + 1) * 128],
                                        identV[:])
                    nc.vector.tensor_copy(vprime[0][:, j, 0:HD], pst[:, 0:HD])
                    nc.vector.tensor_copy(vprime[1][:, j, 0:HD],
                                          pst[:, HD:2 * HD])

                # ---- attention: both heads interleaved through one
                # S(+mask) -> exp -> AV pipeline; prior chunk's proj is
                # inserted mid-stream to cover the AV drain stall ----
                njt = 4 * (c + 1)
                po = [psO.tile([HD + 1, 512], F32, tag=f'o{h}', name=f'po{h}')
                      for h in range(HPC)]
                ets = [[], []]

                def emit_av(h, j):
                    nc.tensor.matmul(
                        po[h][:], vprime[h][:, j, :], ets[h][j][:],
                        start=(j == 0), stop=(j == njt - 1))

                for j in range(njt):
                    for h in range(HPC):
                        ps = psS.tile([128, 512], F32, tag='s')
                        m = j - 4 * c
                        nc.tensor.matmul(
                            ps[:], khp[h][:, j * 128:(j + 1) * 128],
                            qhp[h][:, cs], start=True, stop=(m < 0))
                        if m >= 0:  # diagonal tile: additive causal mask
                            nc.tensor.matmul(ps[:], identR[:],
                                             maskadd[:, m, :],
                                             start=False, stop=True)
                        et = estrip.tile([128, 512], BF16)
                        nc.scalar.activation(et[:], ps[:],
                                             mybir.ActivationFunctionType.Exp,
                                             scale=float(1.0 / np.sqrt(HD)))
                        ets[h].append(et)
                        if j >= LAG:
                            emit_av(h, j - LAG)
                    if j == 1 and pending_proj:
                        emit_proj(pending_proj.pop())
                for j in range(max(0, njt - LAG), njt):
                    for h in range(HPC):
                        emit_av(h, j)
                for h in range(HPC):
                    # normalize: yhat = po[:64] * (1/po[64]) broadcast
                    # (copy denom to partition 0 first: the custom-DVE
                    # reciprocal does not honor input partition offsets)
                    dcp = small.tile([1, 512], F32, tag="dcp")
                    nc.scalar.copy(dcp[:], po[h][HD:HD + 1, :])
                    rec = small.tile([1, 512], F32, tag="rec")
                    nc.vector.reciprocal_approx_fast(rec[:], dcp[:])
                    rb = small.tile([HD, 512], F32, tag="recb")
                    nc.gpsimd.partition_broadcast(rb[:], rec[:])
                    nc.vector.tensor_mul(yhat[h * HD:(h + 1) * HD, cs],
                                         po[h][0:HD, :], rb[:])
                pending_proj.append(c)
            emit_proj(pending_proj.pop())

    nc.compile()
    _nc_cache["attn"] = nc
    return nc


# --------------------------------------------------------------------------
# Launch B: MoE expert (1 expert per core, host-dispatched tokens)
# --------------------------------------------------------------------------

def _chunks(cap):
    # split into equal-ish chunks <= 512 (PSUM bank limit). Equal widths
    # beat (512, small): per-matmul ldweights overhead dominates small
    # moving dims.
    n = (cap + 511) // 512
    base = cap // n
    ch = []
    off = 0
    for i in range(n):
        w = base + (1 if i < cap - base * n else 0)
        ch.append((off, w))
        off += w
    return ch


def build_moe(cap):
    key = ("moe", cap)
    if key in _nc_cache:
        return _nc_cache[key]
    nc = bacc.Bacc("TRN2", target_bir_lowering=False, debug=False,
                   num_devices=NCORES)

    NKC = C // 128    # 8
    NI = HFF // 128   # 32
    NJ = C // 128     # 8
    CH = _chunks(cap)
    DR = mybir.MatmulPerfMode.DoubleRow

    d_xgT = nc.dram_tensor("xgT", [C, cap], F8, kind="ExternalInput")
    # host-pretiled layouts: [block, 128p, k, 128] with contiguous rows;
    # weights pre-scaled by S_W (wg, wd) / S_U (wu) and cast to fp8e4.
    d_wg4 = nc.dram_tensor("wg4", [NI, 128, NKC, 128], F8, kind="ExternalInput")
    d_wu4 = nc.dram_tensor("wu4", [NI, 128, NKC, 128], F8, kind="ExternalInput")
    d_wd4 = nc.dram_tensor("wd4", [NJ, 128, NI, 128], F8, kind="ExternalInput")
    d_yT = nc.dram_tensor("yT", [C, cap], F32, kind="ExternalOutput")

    with tile.TileContext(nc) as tc:
        with tc.tile_pool(name="xg", bufs=1) as xgp, \
             tc.tile_pool(name="hsb", bufs=1) as hsbp, \
             tc.tile_pool(name="hst", bufs=3) as hstp, \
             tc.tile_pool(name="wload", bufs=3) as wload, \
             tc.tile_pool(name="wdload", bufs=3) as wdload, \
             tc.tile_pool(name="ob", bufs=3) as obp, \
             tc.tile_pool(name="psG", bufs=3, space="PSUM") as psG, \
             tc.tile_pool(name="psY", bufs=2, space="PSUM") as psY:

            # xg as one [128, k, n] tile so DoubleRow can take k-pair slices
            xg = xgp.tile([128, NKC, cap], F8)
            nc.sync.dma_start(xg[:], d_xgT.ap().rearrange(
                "(ko p) n -> p ko n", p=128))

            hsb = hsbp.tile([128, NI, cap], F8)

            # Phase 1: h = silu(wg.T@xg) * (wu.T@xg); fp8 DoubleRow over
            # k-tile pairs (contraction 256/instr).
            wdts = {}
            for i in range(NI):
                wgt = wload.tile([128, NKC, 128], F8, tag="wg")
                nc.sync.dma_start(wgt[:], d_wg4.ap()[i])
                wut = wload.tile([128, NKC, 128], F8, tag="wu")
                nc.sync.dma_start(wut[:], d_wu4.ap()[i])
                if i >= NI - 2:  # prefetch phase-2 weights under phase-1 tail
                    j = i - (NI - 2)
                    wdts[j] = wdload.tile([128, NI, 128], F8, tag="wd",
                                          name=f"wdpre{j}")
                    nc.sync.dma_start(wdts[j][:], d_wd4.ap()[j])
                for (off, n) in CH:
                    pg = psG.tile([128, 512], F32, tag="pg")
                    pu = psG.tile([128, 512], F32, tag="pu")
                    for t in range(NKC // 2):
                        nc.tensor.matmul(pg[:, :n], wgt[:, 2 * t:2 * t + 2, :],
                                         xg[:, 2 * t:2 * t + 2, off:off + n],
                                         start=(t == 0), stop=(t == NKC // 2 - 1),
                                         perf_mode=DR)
                    for t in range(NKC // 2):
                        nc.tensor.matmul(pu[:, :n], wut[:, 2 * t:2 * t + 2, :],
                                         xg[:, 2 * t:2 * t + 2, off:off + n],
                                         start=(t == 0), stop=(t == NKC // 2 - 1),
                                         perf_mode=DR)
                    hs = hstp.tile([128, 512], F32)
                    nc.scalar.activation(hs[:, :n], pg[:, :n],
                                         mybir.ActivationFunctionType.Silu,
                                         scale=float(1.0 / S_W))
                    # hsb = silu(g) * (S_U*u) = S_H*h  (S_H == S_U)
                    nc.vector.tensor_mul(hsb[:, i, off:off + n],
                                         hs[:, :n], pu[:, :n])

            # Phase 2: yT[j] = sum_i wd4[j][:, i].T @ h[i]; DR over i-pairs
            for j in range(NJ):
                if j in wdts:
                    wdt = wdts.pop(j)
                else:
                    wdt = wdload.tile([128, NI, 128], F8, tag="wd")
                    nc.sync.dma_start(wdt[:], d_wd4.ap()[j])
                for (off, n) in CH:
                    py = psY.tile([128, 512], F32)
                    for t in range(NI // 2):
                        nc.tensor.matmul(py[:, :n], wdt[:, 2 * t:2 * t + 2, :],
                                         hsb[:, 2 * t:2 * t + 2, off:off + n],
                                         start=(t == 0), stop=(t == NI // 2 - 1),
                                         perf_mode=DR)
                    ob = obp.tile([128, 512], F32)
                    if j % 2 == 0:
                        nc.vector.tensor_copy(ob[:, :n], py[:, :n])
                    else:
                        nc.scalar.copy(ob[:, :n], py[:, :n])
                    nc.sync.dma_start(
                        d_yT.ap()[j * 128:(j + 1) * 128, off:off + n],
                        ob[:, :n])

    nc.compile()
    _nc_cache[key] = nc
    return nc


# --------------------------------------------------------------------------
# Host orchestration
# --------------------------------------------------------------------------

def _rope_tables():
    inv_freq = 1.0 / (10000.0 ** (np.arange(0, HD, 2, dtype=np.float32) / HD))
    t = np.arange(T, dtype=np.float32)
    freqs = np.einsum("i,j->ij", t, inv_freq).astype(np.float32)   # [T, 32]
    emb = np.concatenate([freqs, freqs], axis=-1)                   # [T, 64]
    cos = np.cos(emb).astype(np.float32)
    sin = np.sin(emb).astype(np.float32)
    cosT = np.ascontiguousarray(cos.T)                              # [64, T]
    # stabA pairs with the partition-swapped operand: d<32 -> -sin, d>=32 -> +sin
    sinA = np.empty((HD, T), np.float32)
    sinA[:32] = -sin.T[:32]
    sinA[32:] = sin.T[32:]
    ctab = np.concatenate([cosT] * HPC, axis=0)                     # [128, T]
    stab = np.concatenate([sinA] * HPC, axis=0)
    return ctab, stab


def _causal_masks():
    # additive mask[m, p, f] = 0 where query f sees key (p + 128*m) within
    # the diagonal 512-block, else -1e30 (absorbs s in fp32, exp -> 0).
    f = np.arange(512)[None, :]
    p = np.arange(128)[:, None]
    m4 = np.stack([np.where(f >= p + 128 * m, 0.0, -1e30)
                   for m in range(4)]).astype(np.float32)            # [4,128,512]
    return m4


def _run(nc, in_maps, trace=False, tmpdir=None):
    return run_bass_kernel_spmd(nc, in_maps, list(range(NCORES)),
                                trace=trace, tmpdir=tmpdir)


def kernel(x, norm1_w, norm2_w, qkv_w, proj_w, router_w, wg, wu, wd,
           _trace=False, _stats=None):
    x = np.asarray(x, np.float32)
    B = x.shape[0]
    xf = x.reshape(T, C)

    # ---- host: rms_norm 1 (norm1_w folded into qkv weights) ----
    ms = np.mean(xf * xf, axis=-1, keepdims=True)
    xhat = xf / np.sqrt(ms + EPS)
    xhatT = np.ascontiguousarray(xhat.T)                    # [C, T]

    ctab, stab = _rope_tables()
    masks = _causal_masks()

    qkv_s = (np.asarray(qkv_w, np.float32) * np.asarray(norm1_w, np.float32)[None, :])
    proj = np.asarray(proj_w, np.float32)

    nc_a = build_attention()
    ident = np.eye(128, dtype=np.float32)
    in_maps = []
    for core in range(NCORES):
        h0 = core * HPC
        rows = []
        for g in range(3):  # q, k, v
            rows.append(qkv_s[g * C + h0 * HD: g * C + (h0 + HPC) * HD, :])
        wqkv_c = np.ascontiguousarray(np.concatenate(rows, axis=0).T)  # [C, 384]
        wproj_c = np.ascontiguousarray(proj[:, h0 * HD:(h0 + HPC) * HD].T)  # [128, C]
        in_maps.append({
            "xhatT": xhatT, "wqkv": wqkv_c, "wproj": wproj_c,
            "ctab": ctab, "stab": stab, "mask": masks,
            "identV": ident, "identR": ident,
        })
    res_a = _run(nc_a, in_maps, trace=_trace,
                 tmpdir="/tmp/trace_attn" if _trace else None)
    attn = np.zeros((T, C), np.float32)
    for core in range(NCORES):
        attn += res_a.results[core]["attn_part"]

    xa = xf + attn

    # ---- host: rms_norm 2 + router + top-2 dispatch ----
    ms2 = np.mean(xa * xa, axis=-1, keepdims=True)
    x2 = (xa / np.sqrt(ms2 + EPS)) * np.asarray(norm2_w, np.float32)[None, :]
    logits = x2 @ np.asarray(router_w, np.float32).T        # [T, E]
    topi = np.argsort(-logits, axis=-1)[:, :2]              # [T, 2]
    topv = np.take_along_axis(logits, topi, axis=-1)
    mx = topv.max(axis=-1, keepdims=True)
    ex = np.exp(topv - mx)
    wts = ex / ex.sum(axis=-1, keepdims=True)               # [T, 2]

    idxs, gts = [], []
    for e in range(E):
        sel = np.nonzero((topi == e).any(axis=-1))[0]
        gsel = np.where(topi[sel, 0] == e, wts[sel, 0], wts[sel, 1])
        idxs.append(sel)
        gts.append(gsel.astype(np.float32))
    maxload = max(len(s) for s in idxs)
    cap = max(256, ((maxload + 127) // 128) * 128)

    import ml_dtypes
    F8NP = ml_dtypes.float8_e4m3

    nc_b = build_moe(cap)
    NI, NJ, NKC = HFF // 128, C // 128, C // 128
    in_maps_b = []
    for e in range(E):
        xgT = np.zeros((C, cap), F8NP)
        xgT[:, :len(idxs[e])] = x2[idxs[e]].T.astype(F8NP)
        wg_e = np.asarray(wg[e], np.float32) * S_W
        wu_e = np.asarray(wu[e], np.float32) * S_U
        wd_e = np.asarray(wd[e], np.float32) * S_W
        in_maps_b.append({
            "xgT": xgT,
            "wg4": np.ascontiguousarray(
                wg_e.reshape(NI, 128, NKC, 128).transpose(0, 3, 2, 1)
            ).astype(F8NP),
            "wu4": np.ascontiguousarray(
                wu_e.reshape(NI, 128, NKC, 128).transpose(0, 3, 2, 1)
            ).astype(F8NP),
            "wd4": np.ascontiguousarray(
                wd_e.reshape(NJ, 128, NI, 128).transpose(0, 3, 2, 1)
            ).astype(F8NP),
        })
    res_b = _run(nc_b, in_maps_b, trace=_trace,
                 tmpdir="/tmp/trace_moe" if _trace else None)

    out = xa.copy()
    for e in range(E):
        yT = res_b.results[e]["yT"]                          # [C, cap]
        n = len(idxs[e])
        out[idxs[e]] += yT[:, :n].T * (gts[e] / (S_W * S_H))[:, None]

    if _stats is not None:
        _stats["attn_ns"] = res_a.exec_time_ns
        _stats["moe_ns"] = res_b.exec_time_ns
        _stats["cap"] = cap
    return out.reshape(B, T, C)

